# revision 30
# baseline (speedup 1.0000x reference)
"""Trainium2 Bass kernel for DeepKernelRegressionModel (v2).

Math (per core, X sharded by rows across 8 cores):
  Xf = MLP(X), Yf = MLP(Y)                 (3-layer relu MLP, H=32)
  K[i,m] = exp(Xf_i . Yf_m - |Yf_m|^2/2)   (x-norm term cancels in the
                                            normalized ratio, so skip it)
  out = (K @ Y_target) / (K @ 1)

Design:
  - Y loaded in 8 batched DMAs as [128, 512] tiles, PE-transposed two
    m-tiles per transpose into yT [128, 4096] (even tiles on partitions
    0-63, odd on 64-127).
  - Y-MLP runs 4-way stacked; its relu output yfs [128, 2048] is read
    DIRECTLY as mm1's stationary operand (no assembled yft, no SP DMA
    assembly).
  - y-norms are computed as per-m-tile [128,1] bias COLUMNS via tiny
    matmuls (sqy-slice^T @ -0.5) and applied as the exp activation bias.
  - exp engine per m-tile is configurable: 'A' = exact exp on ACT (f32
    output), 'P'/'D' = Schraudolph bf16 bit-trick on Pool/DVE (one
    tensor_scalar op writing int16 bits of a bf16 exp approximation).
  - mm2 contracts exp tiles with Z = [Y_target, 1, pad] (ZP=16 cols) in
    matching dtype (f32r or bf16); m-tiles are visited in an order that
    rotates mm1 across all 4 PE row-groups for tile concurrency.
"""

import os
import numpy as np
from contextlib import ExitStack

import concourse.bass as bass
import concourse.tile as tile
from concourse import bacc, mybir

FP = mybir.dt.float32
FPR = mybir.dt.float32r
BF = mybir.dt.bfloat16
I16 = mybir.dt.int16
AF = mybir.ActivationFunctionType
ALU = mybir.AluOpType

D, H, T = 64, 32, 8
ZP = 16          # Z columns: Y_target(8) + ones + pad
N_CORES = 8

LN2 = 0.6931471805599453
EXP_S = 128.0 / LN2          # bf16 schraudolph scale
SIGMA = 0.058
EXP_B = (127.0 - SIGMA) * 128.0

# exp-engine pattern over reordered m-tile position (period 8):
# 'A' exact ACT, 'P' Pool bit-trick, 'D' DVE bit-trick
PATTERN = os.environ.get("DKR_PATTERN", "AAAAAAAA")


def mt_order(MT):
    """Visit order rotating mm1 row-groups 0,1,2,3. Octet pair (16 tiles):
    [16a+2j, 16a+2j+1, 16a+8+2j, 16a+8+2j+1] has cg 0,1,2,3."""
    order = []
    a = 0
    while 16 * a < MT:
        hi = 16 * a + 8 < MT
        for j in range(4):
            order.append(16 * a + 2 * j)
            order.append(16 * a + 2 * j + 1)
            if hi:
                order.append(16 * a + 8 + 2 * j)
                order.append(16 * a + 8 + 2 * j + 1)
        a += 1
    assert sorted(order) == list(range(MT))
    return order


def build_nc(n_sh, m_total, use_f32r=True, pattern=None, iters=1,
             split_waits=True):
    assert n_sh % 1024 == 0 and m_total % 2048 == 0
    MT = m_total // 128     # m-tiles
    NCH = m_total // 512    # MLP chunks
    CCY = NCH // 4
    NYD = m_total // 1024   # batched Y DMAs
    IC = n_sh // 512
    ICW = 512
    NXT = n_sh // 128       # x tiles
    pattern = pattern or PATTERN

    def r(ap):
        return ap.bitcast(FPR) if use_f32r else ap

    nc = bacc.Bacc("TRN2", target_bir_lowering=False, debug=False,
                   num_devices=N_CORES)

    Xd = nc.dram_tensor("X", [n_sh, D], FP, kind="ExternalInput").ap()
    Yd = nc.dram_tensor("Y", [m_total, D], FP, kind="ExternalInput").ap()
    Zfd = nc.dram_tensor("Zf", [m_total, ZP], FP, kind="ExternalInput").ap()
    Zbd = nc.dram_tensor("Zb", [m_total, ZP], BF, kind="ExternalInput").ap()
    WBd = nc.dram_tensor("WB", [128, 227], FP, kind="ExternalInput").ap()
    OUTd = nc.dram_tensor("out", [n_sh, T], FP, kind="ExternalOutput").ap()

    with tile.TileContext(nc) as tc, ExitStack() as octx:
        loop_cm = tc.For_i(0, iters, name="bench") if iters > 1 else None
        if loop_cm is not None:
            octx.enter_context(loop_cm)
        with ExitStack() as ctx:
            const = ctx.enter_context(tc.tile_pool(name="const", bufs=1))
            big = ctx.enter_context(tc.tile_pool(name="big", bufs=1))

            wb = const.tile([128, 227], FP)
            nc.sync.dma_start(r(wb[:]), r(WBd[:]))
            w1s = wb[:, 0:32]
            w2s = wb[:, 32:64]
            w3s = wb[:, 64:96]
            bs = wb[:, 96:99]
            ident = wb[:, 99:227]
            nh = const.tile([128, 1], FP)
            nc.gpsimd.memset(nh[:], -0.5)

            ztf = const.tile([128, MT * ZP], FP)
            nc.gpsimd.dma_start(
                r(ztf.rearrange("p (t c) -> p t c", c=ZP)),
                r(Zfd.rearrange("(t p) c -> p t c", p=128)),
            )
            use_bf = True
            ztb = const.tile([128, MT * ZP], BF)
            nc.scalar.dma_start(
                ztb.rearrange("p (t c) -> p t c", c=ZP),
                Zbd.rearrange("(t p) c -> p t c", p=128),
            )

            yT = big.tile([128, m_total // 2], FP)   # packed transposed Y
            xT = big.tile([128, n_sh // 2], FP)
            yfs = big.tile([128, m_total // 4], FP)  # MLP(Y)^T, 4-way stacked
            xft = big.tile([128, n_sh], FP)          # MLP(X)^T, replicated x4
            ynb = big.tile([128, MT], FP)            # -|Yf|^2/2 bias columns
            ynb2 = big.tile([128, MT], FP)           # scaled for bit-trick

            # ---------- phase A: load + transpose (+ X MLP early) ----------
            with (
                tc.tile_pool(name="tp_psum", bufs=2, space="PSUM") as tpp,
                tc.tile_pool(name="raw", bufs=2) as rawp,
                tc.tile_pool(name="xp", bufs=2, space="PSUM") as xpp,
                tc.tile_pool(name="xp3", bufs=1, space="PSUM") as xpp3,
                tc.tile_pool(name="xacts", bufs=2) as xactp,
            ):
                xraw = rawp.tile([128, 512], FP, tag="xraw")
                nc.sync.dma_start(
                    r(xraw.rearrange("p (t c) -> p t c", c=D)),
                    r(Xd.rearrange("(t p) c -> p t c", p=128)),
                )
                tp = tpp.tile([128, 512], FP, tag="tp")
                for j in range(4):
                    nc.tensor.transpose(r(tp[:, 128 * j:128 * j + 128]),
                                        r(xraw[:, 128 * j:128 * j + 128]),
                                        r(ident))
                nc.vector.tensor_copy(r(xT[:]), tp[:])

                dma_engines = [nc.sync, nc.scalar, nc.sync, nc.scalar]
                for g in range(NYD // 2):
                    yraw = rawp.tile([128, 1024], FP, tag="raw")
                    dma_engines[g % 4].dma_start(
                        r(yraw.rearrange("p (t c) -> p t c", c=D)),
                        r(Yd[2048 * g:2048 * g + 2048, :].rearrange(
                            "(t p) c -> p t c", p=128)),
                    )
                    for h in range(2):
                        tp = tpp.tile([128, 512], FP, tag="tp")
                        for j in range(4):
                            nc.tensor.transpose(
                                r(tp[:, 128 * j:128 * j + 128]),
                                r(yraw[:, 512 * h + 128 * j:
                                        512 * h + 128 * j + 128]),
                                r(ident))
                        nc.vector.tensor_copy(
                            r(yT[:, 1024 * g + 512 * h:1024 * g + 512 * h + 512]),
                            tp[:])

                # X MLP (f32r, flat rows 0-31), interleaved with Y loads
                hx1 = xpp.tile([H, n_sh], FP, tag="hx")
                for half in range(2):
                    nc.tensor.matmul(
                        hx1[0:32, 512 * half:512 * half + 512],
                        tile_position=(64 * half, 0),
                        lhsT=r(w1s[64 * half:64 * half + 64, :]),
                        rhs=r(xT[64 * half:64 * half + 64, :]),
                        start=True, stop=True, skip_group_check=True)
                hx1s = xactp.tile([H, n_sh], FP, tag="hxs")
                nc.scalar.activation(r(hx1s[:]), hx1[:], AF.Relu,
                                      bias=bs[0:H, 0:1])
                hx2 = xpp.tile([H, n_sh], FP, tag="hx")
                for half in range(2):
                    nc.tensor.matmul(
                        hx2[0:32, 512 * half:512 * half + 512],
                        tile_position=(0, 0),
                        lhsT=r(w2s[0:32, :]),
                        rhs=r(hx1s[0:32, 512 * half:512 * half + 512]),
                        start=True, stop=True, skip_group_check=True)
                hx2s = xactp.tile([H, n_sh], FP, tag="hxs")
                nc.vector.tensor_scalar(r(hx2s[:]), hx2[:], bs[0:H, 1:2], 0.0,
                                        op0=ALU.add, op1=ALU.max)
                hx3 = xpp3.tile([H, n_sh], FP, tag="hx3")
                for half in range(2):
                    nc.tensor.matmul(
                        hx3[0:32, 512 * half:512 * half + 512],
                        tile_position=(0, 0),
                        lhsT=r(w3s[0:32, :]),
                        rhs=r(hx2s[0:32, 512 * half:512 * half + 512]),
                        start=True, stop=True, skip_group_check=True)
                nc.vector.tensor_scalar(r(xft[0:32, :]), hx3[0:32, :],
                                        bs[0:H, 2:3], 0.0,
                                        op0=ALU.add, op1=ALU.max)
                for gg in range(1, 4):
                    nc.gpsimd.dma_start(r(xft[32 * gg:32 * gg + 32, :]),
                                        r(xft[0:32, :]))

            def yfs_slice(mt):
                ch = 2 * (mt // 8) + (mt % 8) % 2
                j = (mt % 8) // 2
                cg, cc = ch % 4, ch // 4
                col = 512 * cc + 128 * j
                return cg, yfs[32 * cg:32 * cg + 32, col:col + 128]

            def sqy_slice(mt):
                ch = 2 * (mt // 8) + (mt % 8) % 2
                j = (mt % 8) // 2
                cg, cc = ch % 4, ch // 4
                col = 512 * cc + 128 * j
                return cg, sqy[32 * cg:32 * cg + 32, col:col + 128]

            # ---------- phase B: Y MLP ----------
            # L1/L2 are f32r, which the ISA only allows at column-group 0,
            # so they emit flat [32, m] rows 0-31. L3 is plain fp32 (legal
            # with column groups) and emits the 4-way partition-stacked yfs
            # that mm1's rotating row-groups read directly.
            sqyp = ctx.enter_context(tc.tile_pool(name="sqy", bufs=1))
            with (
                tc.tile_pool(name="mlp_psum", bufs=2, space="PSUM") as mpp,
                tc.tile_pool(name="l3_psum", bufs=2, space="PSUM") as mpp3,
                tc.tile_pool(name="ynp", bufs=2, space="PSUM") as ynpp,
                tc.tile_pool(name="acts", bufs=1) as actp,
            ):
                h1s = actp.tile([H, m_total], FP, tag="h1s")
                h2s = actp.tile([H, m_total], FP, tag="h2s")
                npass = (NCH + 1) // 2
                for p in range(npass):
                    chs = range(2 * p, min(2 * p + 2, NCH))
                    h1p = mpp.tile([H, 1024], FP, tag="hp")
                    for i, ch in enumerate(chs):
                        q, half = ch // 2, ch % 2
                        nc.tensor.matmul(
                            h1p[:, 512 * i:512 * i + 512],
                            lhsT=r(w1s[64 * half:64 * half + 64, :]),
                            rhs=r(yT[64 * half:64 * half + 64,
                                     512 * q:512 * q + 512]),
                            tile_position=(64 * half, 0),
                            start=True, stop=True, skip_group_check=True)
                    nc.scalar.activation(
                        r(h1s[:, 1024 * p:1024 * p + 512 * len(chs)]),
                        h1p[:, 0:512 * len(chs)], AF.Relu, bias=bs[0:H, 0:1])
                for p in range(npass):
                    chs = range(2 * p, min(2 * p + 2, NCH))
                    h2p = mpp.tile([H, 1024], FP, tag="hp")
                    for i, ch in enumerate(chs):
                        nc.tensor.matmul(
                            h2p[:, 512 * i:512 * i + 512],
                            lhsT=r(w2s[0:32, :]),
                            rhs=r(h1s[0:32, 512 * ch:512 * ch + 512]),
                            tile_position=(0, 0),
                            start=True, stop=True, skip_group_check=True)
                    nc.vector.tensor_scalar(
                        r(h2s[:, 1024 * p:1024 * p + 512 * len(chs)]),
                        h2p[:, 0:512 * len(chs)], bs[0:H, 1:2], 0.0,
                        op0=ALU.add, op1=ALU.max)
                # L3: fp32, col-grouped into the stacked layout, per-cc
                sqy = sqyp.tile([128, 512 * CCY], FP, tag="sqy")
                for cc in range(CCY):
                    h3p = mpp3.tile([128, 512], FP, tag="h3p")
                    for cg in range(4):
                        ch = 4 * cc + cg
                        nc.tensor.matmul(
                            h3p[32 * cg:32 * cg + 32, :],
                            tile_position=(0, 32 * cg),
                            lhsT=w3s[0:32, :],
                            rhs=h2s[0:32, 512 * ch:512 * ch + 512],
                            start=True, stop=True, skip_group_check=True)
                    nc.vector.tensor_scalar(
                        r(yfs[:, 512 * cc:512 * cc + 512]),
                        h3p[:], bs[:, 2:3], 0.0, op0=ALU.add, op1=ALU.max)
                    nc.vector.tensor_mul(sqy[:, 512 * cc:512 * cc + 512],
                                         yfs[:, 512 * cc:512 * cc + 512],
                                         yfs[:, 512 * cc:512 * cc + 512])
                    ynp = ynpp.tile([128, 16], FP, tag="ynp")
                    mts = [mt for mt in range(16 * cc, min(16 * cc + 16, MT))]
                    for kk, mt in enumerate(mts):
                        scg, sl = sqy_slice(mt)
                        nc.tensor.matmul(
                            ynp[:, kk:kk + 1],
                            tile_position=(32 * scg, 0),
                            lhsT=sl, rhs=nh[32 * scg:32 * scg + 32, :],
                            start=True, stop=True, skip_group_check=True)
                    nc.vector.tensor_copy(ynb[:, 16 * cc:16 * cc + len(mts)],
                                          ynp[:, 0:len(mts)])

            if use_bf:
                nc.vector.tensor_scalar(ynb2[:], ynb[:], float(EXP_S),
                                        float(EXP_B), op0=ALU.mult,
                                        op1=ALU.add)

            # ---------- main loop ----------
            # Both i-chunks processed per m-tile: one [128, 1024] exp per
            # tile (single bias), one weight load per mm1 pair, and two
            # long-lived accumulators.
            order = mt_order(MT)
            with (
                tc.tile_pool(name="gbuf", bufs=2, space="PSUM") as gpool,
                tc.tile_pool(name="accp", bufs=1, space="PSUM") as apool,
                tc.tile_pool(name="ebuf", bufs=3) as epool,
                tc.tile_pool(name="fin", bufs=2) as finp,
            ):
                accs = []
                for ic in range(IC):
                    acc = apool.tile([128, ICW], FP, tag=f"acc{ic}")
                    accs.append(acc)
                for k, mt in enumerate(order):
                    eng = pattern[k % len(pattern)]
                    cg, ysl = yfs_slice(mt)
                    gp = gpool.tile([128, IC * ICW], FP, tag="g")
                    for ic in range(IC):
                        nc.tensor.matmul(
                            gp[:, ICW * ic:ICW * ic + ICW],
                            tile_position=(32 * cg, 0),
                            lhsT=r(ysl),
                            rhs=r(xft[32 * cg:32 * cg + 32,
                                      ICW * ic:ICW * ic + ICW]),
                            start=True, stop=True, skip_group_check=True)
                    if eng == "A":
                        eb = epool.tile([128, IC * ICW], BF, tag="ef")
                        nc.scalar.activation(eb[:], gp[:], AF.Exp,
                                             bias=ynb[:, mt:mt + 1])
                        lz = ztb[:, ZP * mt:ZP * mt + ZP]
                        ebv = eb[:]
                    else:
                        eb = epool.tile([128, IC * ICW], BF, tag="eb")
                        nc.vector.tensor_scalar(eb[:].bitcast(I16), gp[:],
                                                float(EXP_S),
                                                ynb2[:, mt:mt + 1],
                                                op0=ALU.mult, op1=ALU.add)
                        lz = ztb[:, ZP * mt:ZP * mt + ZP]
                        ebv = eb[:]
                    for ic in range(IC):
                        nc.tensor.matmul(
                            accs[ic][0:ZP, :],
                            tile_position=(0, 0),
                            lhsT=lz,
                            rhs=ebv[:, ICW * ic:ICW * ic + ICW],
                            start=(k == 0), stop=(k == MT - 1),
                            skip_group_check=True)
                otm = apool.tile([128, 8 * ZP], FP, tag="ot")
                for ic in range(IC):
                    acc_s = finp.tile([ZP, ICW], FP, tag="accs")
                    nc.vector.tensor_copy(acc_s[:], accs[ic][0:ZP, :])
                    ot = otm[:, 4 * ZP * ic:4 * ZP * ic + 4 * ZP]
                    for q in range(4):
                        nc.tensor.matmul(
                            ot[:, ZP * q:ZP * q + ZP],
                            tile_position=(0, 0),
                            lhsT=acc_s[0:ZP, 128 * q:128 * q + 128],
                            rhs=ident[0:ZP, 0:ZP],
                            is_transpose=True,
                            start=(q == 0), stop=(q == 3),
                            skip_group_check=True)
                    for q in range(4):
                        rec = finp.tile([128, 1], FP, tag="rec")
                        nc.vector.reciprocal(rec[:],
                                             ot[:, ZP * q + T:ZP * q + T + 1])
                        res = finp.tile([128, T], FP, tag="res")
                        nc.vector.tensor_scalar_mul(res[:],
                                                    ot[:, ZP * q:ZP * q + T],
                                                    rec[:])
                        row = 256 * q + 128 * ic
                        (nc.sync if ic == 0 else nc.gpsimd).dma_start(
                            OUTd[row:row + 128, :], res[:])
    nc.compile()
    return nc


def make_in_maps(X, Y, Y_target, W1, b1, W2, b2, W3, b3, n_cores=N_CORES):
    import ml_dtypes

    f = lambda a: np.ascontiguousarray(np.asarray(a, dtype=np.float32))
    X, Y, Y_target = f(X), f(Y), f(Y_target)
    W1, W2, W3 = f(W1), f(W2), f(W3)
    b1, b2, b3 = f(b1), f(b2), f(b3)
    m_total = Y.shape[0]
    n_sh = X.shape[0] // n_cores
    Zf = np.zeros((m_total, ZP), np.float32)
    Zf[:, :T] = Y_target
    Zf[:, T] = 1.0
    WB = np.zeros((128, 227), np.float32)
    WB[0:64, 0:32] = W1
    WB[64:128, 0:32] = W1
    WB[:, 32:64] = np.tile(W2, (4, 1))
    WB[:, 64:96] = np.tile(W3, (4, 1))
    WB[:, 96] = np.tile(b1, 4)
    WB[:, 97] = np.tile(b2, 4)
    WB[:, 98] = np.tile(b3, 4)
    WB[:, 99:227] = np.eye(128, dtype=np.float32)
    common = dict(
        Y=Y, Zf=Zf, Zb=Zf.astype(ml_dtypes.bfloat16),
        WB=np.ascontiguousarray(WB),
    )
    return [dict(common, X=X[c * n_sh:(c + 1) * n_sh]) for c in range(n_cores)]


_NC_CACHE = {}


def _get_nc(n_sh, m_total):
    key = (n_sh, m_total)
    if key not in _NC_CACHE:
        use_f32r = os.environ.get("DKR_F32R", "1") == "1"
        _NC_CACHE[key] = build_nc(n_sh, m_total, use_f32r=use_f32r)
    return _NC_CACHE[key]


def kernel(X, Y, Y_target, W1, b1, W2, b2, W3, b3):
    from concourse.bass_utils import run_bass_kernel_spmd

    in_maps = make_in_maps(X, Y, Y_target, W1, b1, W2, b2, W3, b3)
    n_sh = in_maps[0]["X"].shape[0]
    nc = _get_nc(n_sh, np.asarray(Y).shape[0])
    res = run_bass_kernel_spmd(nc, in_maps, core_ids=list(range(N_CORES)))
    return np.concatenate([res.results[c]["out"] for c in range(N_CORES)], axis=0)


# revision 32
# speedup vs baseline: 1.0445x; 1.0445x over previous
"""Trainium2 Bass kernel for DeepKernelRegressionModel (v2).

Math (per core, X sharded by rows across 8 cores):
  Xf = MLP(X), Yf = MLP(Y)                 (3-layer relu MLP, H=32)
  K[i,m] = exp(Xf_i . Yf_m - |Yf_m|^2/2)   (x-norm term cancels in the
                                            normalized ratio, so skip it)
  out = (K @ Y_target) / (K @ 1)

Design:
  - Y loaded in 8 batched DMAs as [128, 512] tiles, PE-transposed two
    m-tiles per transpose into yT [128, 4096] (even tiles on partitions
    0-63, odd on 64-127).
  - Y-MLP runs 4-way stacked; its relu output yfs [128, 2048] is read
    DIRECTLY as mm1's stationary operand (no assembled yft, no SP DMA
    assembly).
  - y-norms are computed as per-m-tile [128,1] bias COLUMNS via tiny
    matmuls (sqy-slice^T @ -0.5) and applied as the exp activation bias.
  - exp engine per m-tile is configurable: 'A' = exact exp on ACT (f32
    output), 'P'/'D' = Schraudolph bf16 bit-trick on Pool/DVE (one
    tensor_scalar op writing int16 bits of a bf16 exp approximation).
  - mm2 contracts exp tiles with Z = [Y_target, 1, pad] (ZP=16 cols) in
    matching dtype (f32r or bf16); m-tiles are visited in an order that
    rotates mm1 across all 4 PE row-groups for tile concurrency.
"""

import os
import numpy as np
from contextlib import ExitStack

import concourse.bass as bass
import concourse.tile as tile
from concourse import bacc, mybir

FP = mybir.dt.float32
FPR = mybir.dt.float32r
BF = mybir.dt.bfloat16
I16 = mybir.dt.int16
I32 = mybir.dt.int32
AF = mybir.ActivationFunctionType
ALU = mybir.AluOpType

D, H, T = 64, 32, 8
ZP = 16          # Z columns: Y_target(8) + ones + pad
N_CORES = 8

LN2 = 0.6931471805599453
EXP_S = 128.0 / LN2          # bf16 schraudolph scale (legacy)
SIGMA = 0.058
EXP_B = (127.0 - SIGMA) * 128.0
EXP_S32 = float(2.0 ** 23) / LN2     # fp32-bit schraudolph scale
EXP_B32 = (127.0 - SIGMA) * 2.0 ** 23

# exp-engine pattern over reordered m-tile position (period 8):
# 'A' exact ACT, 'P' Pool bit-trick, 'D' DVE bit-trick
PATTERN = os.environ.get("DKR_PATTERN", "AAAAAAAA")


def mt_order(MT):
    """Visit order rotating mm1 row-groups 0,1,2,3. Octet pair (16 tiles):
    [16a+2j, 16a+2j+1, 16a+8+2j, 16a+8+2j+1] has cg 0,1,2,3."""
    order = []
    a = 0
    while 16 * a < MT:
        hi = 16 * a + 8 < MT
        for j in range(4):
            order.append(16 * a + 2 * j)
            order.append(16 * a + 2 * j + 1)
            if hi:
                order.append(16 * a + 8 + 2 * j)
                order.append(16 * a + 8 + 2 * j + 1)
        a += 1
    assert sorted(order) == list(range(MT))
    return order


def build_nc(n_sh, m_total, use_f32r=True, pattern=None, iters=1,
             split_waits=True):
    assert n_sh % 1024 == 0 and m_total % 2048 == 0
    MT = m_total // 128     # m-tiles
    NCH = m_total // 512    # MLP chunks
    CCY = NCH // 4
    NYD = m_total // 1024   # batched Y DMAs
    IC = n_sh // 512
    ICW = 512
    NXT = n_sh // 128       # x tiles
    pattern = pattern or PATTERN

    def r(ap):
        return ap.bitcast(FPR) if use_f32r else ap

    nc = bacc.Bacc("TRN2", target_bir_lowering=False, debug=False,
                   num_devices=N_CORES)

    Xd = nc.dram_tensor("X", [n_sh, D], FP, kind="ExternalInput").ap()
    Yd = nc.dram_tensor("Y", [m_total, D], FP, kind="ExternalInput").ap()
    Zfd = nc.dram_tensor("Zf", [m_total, ZP], FP, kind="ExternalInput").ap()
    Zbd = nc.dram_tensor("Zb", [m_total, ZP], BF, kind="ExternalInput").ap()
    WBd = nc.dram_tensor("WB", [128, 227], FP, kind="ExternalInput").ap()
    OUTd = nc.dram_tensor("out", [n_sh, T], FP, kind="ExternalOutput").ap()

    with tile.TileContext(nc) as tc, ExitStack() as octx:
        loop_cm = tc.For_i(0, iters, name="bench") if iters > 1 else None
        if loop_cm is not None:
            octx.enter_context(loop_cm)
        with ExitStack() as ctx:
            const = ctx.enter_context(tc.tile_pool(name="const", bufs=1))
            big = ctx.enter_context(tc.tile_pool(name="big", bufs=1))

            wb = const.tile([128, 227], FP)
            nc.sync.dma_start(r(wb[:]), r(WBd[:]))
            w1s = wb[:, 0:32]
            w2s = wb[:, 32:64]
            w3s = wb[:, 64:96]
            bs = wb[:, 96:99]
            ident = wb[:, 99:227]
            nh = const.tile([128, 1], FP)
            nc.gpsimd.memset(nh[:], -0.5)

            ztf = const.tile([128, MT * ZP], FP)
            nc.gpsimd.dma_start(
                r(ztf.rearrange("p (t c) -> p t c", c=ZP)),
                r(Zfd.rearrange("(t p) c -> p t c", p=128)),
            )
            use_bf = any(c != "A" for c in pattern)

            yT = big.tile([128, m_total // 2], FP)   # packed transposed Y
            xT = big.tile([128, n_sh // 2], FP)
            yfs = big.tile([128, m_total // 4], FP)  # MLP(Y)^T, 4-way stacked
            xft = big.tile([128, n_sh], FP)          # MLP(X)^T, replicated x4
            ynb = big.tile([128, MT], FP)            # -|Yf|^2/2 bias columns
            ynb2 = big.tile([128, MT], FP)           # scaled for bit-trick

            # ---------- phase A: load + transpose (+ X MLP early) ----------
            with (
                tc.tile_pool(name="tp_psum", bufs=2, space="PSUM") as tpp,
                tc.tile_pool(name="raw", bufs=2) as rawp,
                tc.tile_pool(name="xp", bufs=2, space="PSUM") as xpp,
                tc.tile_pool(name="xp3", bufs=1, space="PSUM") as xpp3,
                tc.tile_pool(name="xacts", bufs=2) as xactp,
            ):
                xraw = rawp.tile([128, 512], FP, tag="xraw")
                nc.sync.dma_start(
                    r(xraw.rearrange("p (t c) -> p t c", c=D)),
                    r(Xd.rearrange("(t p) c -> p t c", p=128)),
                )
                tp = tpp.tile([128, 512], FP, tag="tp")
                for j in range(4):
                    nc.tensor.transpose(r(tp[:, 128 * j:128 * j + 128]),
                                        r(xraw[:, 128 * j:128 * j + 128]),
                                        r(ident))
                nc.vector.tensor_copy(r(xT[:]), tp[:])

                dma_engines = [nc.sync, nc.scalar, nc.sync, nc.scalar]
                for g in range(NYD // 2):
                    yraw = rawp.tile([128, 1024], FP, tag="raw")
                    dma_engines[g % 4].dma_start(
                        r(yraw.rearrange("p (t c) -> p t c", c=D)),
                        r(Yd[2048 * g:2048 * g + 2048, :].rearrange(
                            "(t p) c -> p t c", p=128)),
                    )
                    for h in range(2):
                        tp = tpp.tile([128, 512], FP, tag="tp")
                        for j in range(4):
                            nc.tensor.transpose(
                                r(tp[:, 128 * j:128 * j + 128]),
                                r(yraw[:, 512 * h + 128 * j:
                                        512 * h + 128 * j + 128]),
                                r(ident))
                        nc.vector.tensor_copy(
                            r(yT[:, 1024 * g + 512 * h:1024 * g + 512 * h + 512]),
                            tp[:])

                # X MLP (f32r, flat rows 0-31), interleaved with Y loads
                hx1 = xpp.tile([H, n_sh], FP, tag="hx")
                for half in range(2):
                    nc.tensor.matmul(
                        hx1[0:32, 512 * half:512 * half + 512],
                        tile_position=(64 * half, 0),
                        lhsT=r(w1s[64 * half:64 * half + 64, :]),
                        rhs=r(xT[64 * half:64 * half + 64, :]),
                        start=True, stop=True, skip_group_check=True)
                hx1s = xactp.tile([H, n_sh], FP, tag="hxs")
                nc.scalar.activation(r(hx1s[:]), hx1[:], AF.Relu,
                                      bias=bs[0:H, 0:1])
                hx2 = xpp.tile([H, n_sh], FP, tag="hx")
                for half in range(2):
                    nc.tensor.matmul(
                        hx2[0:32, 512 * half:512 * half + 512],
                        tile_position=(0, 0),
                        lhsT=r(w2s[0:32, :]),
                        rhs=r(hx1s[0:32, 512 * half:512 * half + 512]),
                        start=True, stop=True, skip_group_check=True)
                hx2s = xactp.tile([H, n_sh], FP, tag="hxs")
                nc.vector.tensor_scalar(r(hx2s[:]), hx2[:], bs[0:H, 1:2], 0.0,
                                        op0=ALU.add, op1=ALU.max)
                hx3 = xpp3.tile([H, n_sh], FP, tag="hx3")
                for half in range(2):
                    nc.tensor.matmul(
                        hx3[0:32, 512 * half:512 * half + 512],
                        tile_position=(0, 0),
                        lhsT=r(w3s[0:32, :]),
                        rhs=r(hx2s[0:32, 512 * half:512 * half + 512]),
                        start=True, stop=True, skip_group_check=True)
                nc.vector.tensor_scalar(r(xft[0:32, :]), hx3[0:32, :],
                                        bs[0:H, 2:3], 0.0,
                                        op0=ALU.add, op1=ALU.max)
                for gg in range(1, 4):
                    nc.gpsimd.dma_start(r(xft[32 * gg:32 * gg + 32, :]),
                                        r(xft[0:32, :]))

            def yfs_slice(mt):
                ch = 2 * (mt // 8) + (mt % 8) % 2
                j = (mt % 8) // 2
                cg, cc = ch % 4, ch // 4
                col = 512 * cc + 128 * j
                return cg, yfs[32 * cg:32 * cg + 32, col:col + 128]

            def sqy_slice(mt):
                ch = 2 * (mt // 8) + (mt % 8) % 2
                j = (mt % 8) // 2
                cg, cc = ch % 4, ch // 4
                col = 512 * cc + 128 * j
                return cg, sqy[32 * cg:32 * cg + 32, col:col + 128]

            # ---------- phase B: Y MLP ----------
            # L1/L2 are f32r, which the ISA only allows at column-group 0,
            # so they emit flat [32, m] rows 0-31. L3 is plain fp32 (legal
            # with column groups) and emits the 4-way partition-stacked yfs
            # that mm1's rotating row-groups read directly.
            sqyp = ctx.enter_context(tc.tile_pool(name="sqy", bufs=1))
            with (
                tc.tile_pool(name="mlp_psum", bufs=2, space="PSUM") as mpp,
                tc.tile_pool(name="l3_psum", bufs=2, space="PSUM") as mpp3,
                tc.tile_pool(name="ynp", bufs=2, space="PSUM") as ynpp,
                tc.tile_pool(name="acts", bufs=1) as actp,
            ):
                h1s = actp.tile([H, m_total], FP, tag="h1s")
                h2s = actp.tile([H, m_total], FP, tag="h2s")
                npass = (NCH + 1) // 2
                for p in range(npass):
                    chs = range(2 * p, min(2 * p + 2, NCH))
                    h1p = mpp.tile([H, 1024], FP, tag="hp")
                    for i, ch in enumerate(chs):
                        q, half = ch // 2, ch % 2
                        nc.tensor.matmul(
                            h1p[:, 512 * i:512 * i + 512],
                            lhsT=r(w1s[64 * half:64 * half + 64, :]),
                            rhs=r(yT[64 * half:64 * half + 64,
                                     512 * q:512 * q + 512]),
                            tile_position=(64 * half, 0),
                            start=True, stop=True, skip_group_check=True)
                    nc.scalar.activation(
                        r(h1s[:, 1024 * p:1024 * p + 512 * len(chs)]),
                        h1p[:, 0:512 * len(chs)], AF.Relu, bias=bs[0:H, 0:1])
                for p in range(npass):
                    chs = range(2 * p, min(2 * p + 2, NCH))
                    h2p = mpp.tile([H, 1024], FP, tag="hp")
                    for i, ch in enumerate(chs):
                        nc.tensor.matmul(
                            h2p[:, 512 * i:512 * i + 512],
                            lhsT=r(w2s[0:32, :]),
                            rhs=r(h1s[0:32, 512 * ch:512 * ch + 512]),
                            tile_position=(0, 0),
                            start=True, stop=True, skip_group_check=True)
                    nc.vector.tensor_scalar(
                        r(h2s[:, 1024 * p:1024 * p + 512 * len(chs)]),
                        h2p[:, 0:512 * len(chs)], bs[0:H, 1:2], 0.0,
                        op0=ALU.add, op1=ALU.max)
                # L3: fp32, col-grouped into the stacked layout, per-cc
                sqy = sqyp.tile([128, 512 * CCY], FP, tag="sqy")
                for cc in range(CCY):
                    h3p = mpp3.tile([128, 512], FP, tag="h3p")
                    for cg in range(4):
                        ch = 4 * cc + cg
                        nc.tensor.matmul(
                            h3p[32 * cg:32 * cg + 32, :],
                            tile_position=(0, 32 * cg),
                            lhsT=w3s[0:32, :],
                            rhs=h2s[0:32, 512 * ch:512 * ch + 512],
                            start=True, stop=True, skip_group_check=True)
                    nc.vector.tensor_scalar(
                        r(yfs[:, 512 * cc:512 * cc + 512]),
                        h3p[:], bs[:, 2:3], 0.0, op0=ALU.add, op1=ALU.max)
                    nc.vector.tensor_mul(sqy[:, 512 * cc:512 * cc + 512],
                                         yfs[:, 512 * cc:512 * cc + 512],
                                         yfs[:, 512 * cc:512 * cc + 512])
                    ynp = ynpp.tile([128, 16], FP, tag="ynp")
                    mts = [mt for mt in range(16 * cc, min(16 * cc + 16, MT))]
                    for kk, mt in enumerate(mts):
                        scg, sl = sqy_slice(mt)
                        nc.tensor.matmul(
                            ynp[:, kk:kk + 1],
                            tile_position=(32 * scg, 0),
                            lhsT=sl, rhs=nh[32 * scg:32 * scg + 32, :],
                            start=True, stop=True, skip_group_check=True)
                    nc.vector.tensor_copy(ynb[:, 16 * cc:16 * cc + len(mts)],
                                          ynp[:, 0:len(mts)])

            if use_bf:
                nc.vector.tensor_scalar(ynb2[:], ynb[:], float(EXP_S32),
                                        float(EXP_B32), op0=ALU.mult,
                                        op1=ALU.add)

            # ---------- main loop ----------
            # Both i-chunks processed per m-tile: one [128, 1024] exp per
            # tile (single bias), one weight load per mm1 pair, and two
            # long-lived accumulators.
            order = mt_order(MT)
            with (
                tc.tile_pool(name="gbuf", bufs=2, space="PSUM") as gpool,
                tc.tile_pool(name="accp", bufs=1, space="PSUM") as apool,
                tc.tile_pool(name="ebuf", bufs=3) as epool,
                tc.tile_pool(name="fin", bufs=2) as finp,
            ):
                accs = []
                for ic in range(IC):
                    acc = apool.tile([128, ICW], FP, tag=f"acc{ic}")
                    accs.append(acc)
                for k, mt in enumerate(order):
                    eng = pattern[k % len(pattern)]
                    cg, ysl = yfs_slice(mt)
                    gp = gpool.tile([128, IC * ICW], FP, tag="g")
                    for ic in range(IC):
                        nc.tensor.matmul(
                            gp[:, ICW * ic:ICW * ic + ICW],
                            tile_position=(32 * cg, 0),
                            lhsT=r(ysl),
                            rhs=r(xft[32 * cg:32 * cg + 32,
                                      ICW * ic:ICW * ic + ICW]),
                            start=True, stop=True, skip_group_check=True)
                    if eng == "A":
                        eb = epool.tile([128, IC * ICW], FP, tag="ef")
                        nc.scalar.activation(r(eb[:]), gp[:], AF.Exp,
                                             bias=ynb[:, mt:mt + 1])
                        lz = r(ztf[:, ZP * mt:ZP * mt + ZP])
                        ebv = r(eb[:])
                    else:
                        # int32 Schraudolph: DVE converts (g*S+B) to int32
                        # whose bits are a valid fp32 ~= exp(g+ynb); a Pool
                        # copy re-types it as a rounded f32r producer for mm2
                        ebi = epool.tile([128, IC * ICW], I32, tag="ebi")
                        nc.vector.tensor_scalar(ebi[:], gp[:],
                                                float(EXP_S32),
                                                ynb2[:, mt:mt + 1],
                                                op0=ALU.mult, op1=ALU.add)
                        eb = epool.tile([128, IC * ICW], FP, tag="ef")
                        nc.gpsimd.tensor_copy(r(eb[:]), ebi[:].bitcast(FP))
                        lz = r(ztf[:, ZP * mt:ZP * mt + ZP])
                        ebv = r(eb[:])
                    for ic in range(IC):
                        nc.tensor.matmul(
                            accs[ic][0:ZP, :],
                            tile_position=(0, 0),
                            lhsT=lz,
                            rhs=ebv[:, ICW * ic:ICW * ic + ICW],
                            start=(k == 0), stop=(k == MT - 1),
                            skip_group_check=True)
                otm = apool.tile([128, 8 * ZP], FP, tag="ot")
                for ic in range(IC):
                    acc_s = finp.tile([ZP, ICW], FP, tag="accs")
                    nc.vector.tensor_copy(acc_s[:], accs[ic][0:ZP, :])
                    ot = otm[:, 4 * ZP * ic:4 * ZP * ic + 4 * ZP]
                    for q in range(4):
                        nc.tensor.matmul(
                            ot[:, ZP * q:ZP * q + ZP],
                            tile_position=(0, 0),
                            lhsT=acc_s[0:ZP, 128 * q:128 * q + 128],
                            rhs=ident[0:ZP, 0:ZP],
                            is_transpose=True,
                            start=(q == 0), stop=(q == 3),
                            skip_group_check=True)
                    for q in range(4):
                        rec = finp.tile([128, 1], FP, tag="rec")
                        nc.vector.reciprocal(rec[:],
                                             ot[:, ZP * q + T:ZP * q + T + 1])
                        res = finp.tile([128, T], FP, tag="res")
                        nc.vector.tensor_scalar_mul(res[:],
                                                    ot[:, ZP * q:ZP * q + T],
                                                    rec[:])
                        row = 256 * q + 128 * ic
                        (nc.sync if ic == 0 else nc.gpsimd).dma_start(
                            OUTd[row:row + 128, :], res[:])
    nc.compile()
    return nc


def make_in_maps(X, Y, Y_target, W1, b1, W2, b2, W3, b3, n_cores=N_CORES):
    import ml_dtypes

    f = lambda a: np.ascontiguousarray(np.asarray(a, dtype=np.float32))
    X, Y, Y_target = f(X), f(Y), f(Y_target)
    W1, W2, W3 = f(W1), f(W2), f(W3)
    b1, b2, b3 = f(b1), f(b2), f(b3)
    m_total = Y.shape[0]
    n_sh = X.shape[0] // n_cores
    Zf = np.zeros((m_total, ZP), np.float32)
    Zf[:, :T] = Y_target
    Zf[:, T] = 1.0
    WB = np.zeros((128, 227), np.float32)
    WB[0:64, 0:32] = W1
    WB[64:128, 0:32] = W1
    WB[:, 32:64] = np.tile(W2, (4, 1))
    WB[:, 64:96] = np.tile(W3, (4, 1))
    WB[:, 96] = np.tile(b1, 4)
    WB[:, 97] = np.tile(b2, 4)
    WB[:, 98] = np.tile(b3, 4)
    WB[:, 99:227] = np.eye(128, dtype=np.float32)
    common = dict(
        Y=Y, Zf=Zf, Zb=Zf.astype(ml_dtypes.bfloat16),
        WB=np.ascontiguousarray(WB),
    )
    return [dict(common, X=X[c * n_sh:(c + 1) * n_sh]) for c in range(n_cores)]


_NC_CACHE = {}


def _get_nc(n_sh, m_total):
    key = (n_sh, m_total)
    if key not in _NC_CACHE:
        use_f32r = os.environ.get("DKR_F32R", "1") == "1"
        _NC_CACHE[key] = build_nc(n_sh, m_total, use_f32r=use_f32r)
    return _NC_CACHE[key]


def kernel(X, Y, Y_target, W1, b1, W2, b2, W3, b3):
    from concourse.bass_utils import run_bass_kernel_spmd

    in_maps = make_in_maps(X, Y, Y_target, W1, b1, W2, b2, W3, b3)
    n_sh = in_maps[0]["X"].shape[0]
    nc = _get_nc(n_sh, np.asarray(Y).shape[0])
    res = run_bass_kernel_spmd(nc, in_maps, core_ids=list(range(N_CORES)))
    return np.concatenate([res.results[c]["out"] for c in range(N_CORES)], axis=0)


# revision 33
# speedup vs baseline: 1.1226x; 1.0748x over previous
"""Trainium2 Bass kernel for DeepKernelRegressionModel (v2).

Math (per core, X sharded by rows across 8 cores):
  Xf = MLP(X), Yf = MLP(Y)                 (3-layer relu MLP, H=32)
  K[i,m] = exp(Xf_i . Yf_m - |Yf_m|^2/2)   (x-norm term cancels in the
                                            normalized ratio, so skip it)
  out = (K @ Y_target) / (K @ 1)

Design:
  - Y loaded in 8 batched DMAs as [128, 512] tiles, PE-transposed two
    m-tiles per transpose into yT [128, 4096] (even tiles on partitions
    0-63, odd on 64-127).
  - Y-MLP runs 4-way stacked; its relu output yfs [128, 2048] is read
    DIRECTLY as mm1's stationary operand (no assembled yft, no SP DMA
    assembly).
  - y-norms are computed as per-m-tile [128,1] bias COLUMNS via tiny
    matmuls (sqy-slice^T @ -0.5) and applied as the exp activation bias.
  - exp engine per m-tile is configurable: 'A' = exact exp on ACT (f32
    output), 'P'/'D' = Schraudolph bf16 bit-trick on Pool/DVE (one
    tensor_scalar op writing int16 bits of a bf16 exp approximation).
  - mm2 contracts exp tiles with Z = [Y_target, 1, pad] (ZP=16 cols) in
    matching dtype (f32r or bf16); m-tiles are visited in an order that
    rotates mm1 across all 4 PE row-groups for tile concurrency.
"""

import os
import numpy as np
from contextlib import ExitStack

import concourse.bass as bass
import concourse.tile as tile
from concourse import bacc, mybir

FP = mybir.dt.float32
FPR = mybir.dt.float32r
BF = mybir.dt.bfloat16
I16 = mybir.dt.int16
I32 = mybir.dt.int32
AF = mybir.ActivationFunctionType
ALU = mybir.AluOpType

D, H, T = 64, 32, 8
ZP = 16          # Z columns: Y_target(8) + ones + pad
N_CORES = 8

LN2 = 0.6931471805599453
EXP_S = 128.0 / LN2          # bf16 schraudolph scale (legacy)
SIGMA = 0.058
EXP_B = (127.0 - SIGMA) * 128.0
EXP_S32 = float(2.0 ** 23) / LN2     # fp32-bit schraudolph scale
EXP_B32 = (127.0 - SIGMA) * 2.0 ** 23

# exp-engine pattern over reordered m-tile position (period 8):
# 'A' exact ACT, 'P' Pool bit-trick, 'D' DVE bit-trick
PATTERN = os.environ.get("DKR_PATTERN", "AAAAAAAA")


def mt_order(MT):
    """Visit order rotating mm1 row-groups 0,1,2,3. Octet pair (16 tiles):
    [16a+2j, 16a+2j+1, 16a+8+2j, 16a+8+2j+1] has cg 0,1,2,3."""
    order = []
    a = 0
    while 16 * a < MT:
        hi = 16 * a + 8 < MT
        for j in range(4):
            order.append(16 * a + 2 * j)
            order.append(16 * a + 2 * j + 1)
            if hi:
                order.append(16 * a + 8 + 2 * j)
                order.append(16 * a + 8 + 2 * j + 1)
        a += 1
    assert sorted(order) == list(range(MT))
    return order


def build_nc(n_sh, m_total, use_f32r=True, pattern=None, iters=1,
             split_waits=True):
    assert n_sh % 1024 == 0 and m_total % 2048 == 0
    MT = m_total // 128     # m-tiles
    NCH = m_total // 512    # MLP chunks
    CCY = NCH // 4
    NYD = m_total // 1024   # batched Y DMAs
    IC = n_sh // 512
    ICW = 512
    NXT = n_sh // 128       # x tiles
    pattern = pattern or PATTERN

    def r(ap):
        return ap.bitcast(FPR) if use_f32r else ap

    nc = bacc.Bacc("TRN2", target_bir_lowering=False, debug=False,
                   num_devices=N_CORES)

    Xd = nc.dram_tensor("X", [n_sh, D], FP, kind="ExternalInput").ap()
    Yd = nc.dram_tensor("Y", [m_total, D], FP, kind="ExternalInput").ap()
    Zfd = nc.dram_tensor("Zf", [m_total, ZP], FP, kind="ExternalInput").ap()
    Zbd = nc.dram_tensor("Zb", [m_total, ZP], BF, kind="ExternalInput").ap()
    WBd = nc.dram_tensor("WB", [128, 227], FP, kind="ExternalInput").ap()
    OUTd = nc.dram_tensor("out", [n_sh, T], FP, kind="ExternalOutput").ap()

    with tile.TileContext(nc) as tc, ExitStack() as octx:
        loop_cm = tc.For_i(0, iters, name="bench") if iters > 1 else None
        if loop_cm is not None:
            octx.enter_context(loop_cm)
        with ExitStack() as ctx:
            const = ctx.enter_context(tc.tile_pool(name="const", bufs=1))
            big = ctx.enter_context(tc.tile_pool(name="big", bufs=1))

            wb = const.tile([128, 227], FP)
            nc.sync.dma_start(r(wb[:]), r(WBd[:]))
            w1s = wb[:, 0:32]
            w2s = wb[:, 32:64]
            w3s = wb[:, 64:96]
            bs = wb[:, 96:99]
            ident = wb[:, 99:227]
            nh = const.tile([128, 1], FP)
            nc.gpsimd.memset(nh[:], -0.5)

            ztf = const.tile([128, MT * ZP], FP)
            nc.gpsimd.dma_start(
                r(ztf.rearrange("p (t c) -> p t c", c=ZP)),
                r(Zfd.rearrange("(t p) c -> p t c", p=128)),
            )
            use_bf = any(c != "A" for c in pattern)

            yT = big.tile([128, m_total // 2], FP)   # packed transposed Y
            xT = big.tile([128, n_sh // 2], FP)
            yfs = big.tile([128, m_total // 4], FP)  # MLP(Y)^T, 4-way stacked
            xft = big.tile([128, n_sh], FP)          # MLP(X)^T, replicated x4
            ynb = big.tile([128, MT], FP)            # -|Yf|^2/2 bias columns
            ynb2 = big.tile([128, MT], FP)           # scaled for bit-trick

            # ---------- phase A: load + transpose (+ X MLP early) ----------
            with (
                tc.tile_pool(name="tp_psum", bufs=2, space="PSUM") as tpp,
                tc.tile_pool(name="raw", bufs=2) as rawp,
                tc.tile_pool(name="xp", bufs=2, space="PSUM") as xpp,
                tc.tile_pool(name="xp3", bufs=1, space="PSUM") as xpp3,
                tc.tile_pool(name="xacts", bufs=2) as xactp,
            ):
                xraw = rawp.tile([128, 512], FP, tag="xraw")
                nc.sync.dma_start(
                    r(xraw.rearrange("p (t c) -> p t c", c=D)),
                    r(Xd.rearrange("(t p) c -> p t c", p=128)),
                )
                tp = tpp.tile([128, 512], FP, tag="tp")
                for j in range(4):
                    nc.tensor.transpose(r(tp[:, 128 * j:128 * j + 128]),
                                        r(xraw[:, 128 * j:128 * j + 128]),
                                        r(ident))
                nc.vector.tensor_copy(r(xT[:]), tp[:])

                dma_engines = [nc.sync, nc.scalar, nc.sync, nc.scalar]
                for g in range(NYD // 2):
                    yraw = rawp.tile([128, 1024], FP, tag="raw")
                    dma_engines[g % 4].dma_start(
                        r(yraw.rearrange("p (t c) -> p t c", c=D)),
                        r(Yd[2048 * g:2048 * g + 2048, :].rearrange(
                            "(t p) c -> p t c", p=128)),
                    )
                    for h in range(2):
                        tp = tpp.tile([128, 512], FP, tag="tp")
                        for j in range(4):
                            nc.tensor.transpose(
                                r(tp[:, 128 * j:128 * j + 128]),
                                r(yraw[:, 512 * h + 128 * j:
                                        512 * h + 128 * j + 128]),
                                r(ident))
                        nc.vector.tensor_copy(
                            r(yT[:, 1024 * g + 512 * h:1024 * g + 512 * h + 512]),
                            tp[:])

                # X MLP (f32r, flat rows 0-31), interleaved with Y loads
                hx1 = xpp.tile([H, n_sh], FP, tag="hx")
                for half in range(2):
                    nc.tensor.matmul(
                        hx1[0:32, 512 * half:512 * half + 512],
                        tile_position=(64 * half, 0),
                        lhsT=r(w1s[64 * half:64 * half + 64, :]),
                        rhs=r(xT[64 * half:64 * half + 64, :]),
                        start=True, stop=True, skip_group_check=True)
                hx1s = xactp.tile([H, n_sh], FP, tag="hxs")
                nc.scalar.activation(r(hx1s[:]), hx1[:], AF.Relu,
                                      bias=bs[0:H, 0:1])
                hx2 = xpp.tile([H, n_sh], FP, tag="hx")
                for half in range(2):
                    nc.tensor.matmul(
                        hx2[0:32, 512 * half:512 * half + 512],
                        tile_position=(0, 0),
                        lhsT=r(w2s[0:32, :]),
                        rhs=r(hx1s[0:32, 512 * half:512 * half + 512]),
                        start=True, stop=True, skip_group_check=True)
                hx2s = xactp.tile([H, n_sh], FP, tag="hxs")
                nc.vector.tensor_scalar(r(hx2s[:]), hx2[:], bs[0:H, 1:2], 0.0,
                                        op0=ALU.add, op1=ALU.max)
                hx3 = xpp3.tile([H, n_sh], FP, tag="hx3")
                for half in range(2):
                    nc.tensor.matmul(
                        hx3[0:32, 512 * half:512 * half + 512],
                        tile_position=(0, 0),
                        lhsT=r(w3s[0:32, :]),
                        rhs=r(hx2s[0:32, 512 * half:512 * half + 512]),
                        start=True, stop=True, skip_group_check=True)
                nc.vector.tensor_scalar(r(xft[0:32, :]), hx3[0:32, :],
                                        bs[0:H, 2:3], 0.0,
                                        op0=ALU.add, op1=ALU.max)
                for gg in range(1, 4):
                    nc.gpsimd.dma_start(r(xft[32 * gg:32 * gg + 32, :]),
                                        r(xft[0:32, :]))

            def yfs_slice(mt):
                ch = 2 * (mt // 8) + (mt % 8) % 2
                j = (mt % 8) // 2
                cg, cc = ch % 4, ch // 4
                col = 512 * cc + 128 * j
                return cg, yfs[32 * cg:32 * cg + 32, col:col + 128]

            def sqy_slice(mt):
                ch = 2 * (mt // 8) + (mt % 8) % 2
                j = (mt % 8) // 2
                cg, cc = ch % 4, ch // 4
                col = 512 * cc + 128 * j
                return cg, sqy[32 * cg:32 * cg + 32, col:col + 128]

            # ---------- phase B: Y MLP ----------
            # L1/L2 are f32r, which the ISA only allows at column-group 0,
            # so they emit flat [32, m] rows 0-31. L3 is plain fp32 (legal
            # with column groups) and emits the 4-way partition-stacked yfs
            # that mm1's rotating row-groups read directly.
            sqyp = ctx.enter_context(tc.tile_pool(name="sqy", bufs=1))
            with (
                tc.tile_pool(name="mlp_psum", bufs=2, space="PSUM") as mpp,
                tc.tile_pool(name="l3_psum", bufs=2, space="PSUM") as mpp3,
                tc.tile_pool(name="ynp", bufs=2, space="PSUM") as ynpp,
                tc.tile_pool(name="acts", bufs=1) as actp,
            ):
                h1s = actp.tile([H, m_total], FP, tag="h1s")
                h2s = actp.tile([H, m_total], FP, tag="h2s")
                npass = (NCH + 1) // 2
                for p in range(npass):
                    chs = range(2 * p, min(2 * p + 2, NCH))
                    h1p = mpp.tile([H, 1024], FP, tag="hp")
                    for i, ch in enumerate(chs):
                        q, half = ch // 2, ch % 2
                        nc.tensor.matmul(
                            h1p[:, 512 * i:512 * i + 512],
                            lhsT=r(w1s[64 * half:64 * half + 64, :]),
                            rhs=r(yT[64 * half:64 * half + 64,
                                     512 * q:512 * q + 512]),
                            tile_position=(64 * half, 0),
                            start=True, stop=True, skip_group_check=True)
                    nc.scalar.activation(
                        r(h1s[:, 1024 * p:1024 * p + 512 * len(chs)]),
                        h1p[:, 0:512 * len(chs)], AF.Relu, bias=bs[0:H, 0:1])
                for p in range(npass):
                    chs = range(2 * p, min(2 * p + 2, NCH))
                    h2p = mpp.tile([H, 1024], FP, tag="hp")
                    for i, ch in enumerate(chs):
                        nc.tensor.matmul(
                            h2p[:, 512 * i:512 * i + 512],
                            lhsT=r(w2s[0:32, :]),
                            rhs=r(h1s[0:32, 512 * ch:512 * ch + 512]),
                            tile_position=(0, 0),
                            start=True, stop=True, skip_group_check=True)
                    nc.vector.tensor_scalar(
                        r(h2s[:, 1024 * p:1024 * p + 512 * len(chs)]),
                        h2p[:, 0:512 * len(chs)], bs[0:H, 1:2], 0.0,
                        op0=ALU.add, op1=ALU.max)
                # L3: fp32, col-grouped into the stacked layout, per-cc
                sqy = sqyp.tile([128, 512 * CCY], FP, tag="sqy")
                for cc in range(CCY):
                    h3p = mpp3.tile([128, 512], FP, tag="h3p")
                    for cg in range(4):
                        ch = 4 * cc + cg
                        nc.tensor.matmul(
                            h3p[32 * cg:32 * cg + 32, :],
                            tile_position=(0, 32 * cg),
                            lhsT=w3s[0:32, :],
                            rhs=h2s[0:32, 512 * ch:512 * ch + 512],
                            start=True, stop=True, skip_group_check=True)
                    nc.vector.tensor_scalar(
                        r(yfs[:, 512 * cc:512 * cc + 512]),
                        h3p[:], bs[:, 2:3], 0.0, op0=ALU.add, op1=ALU.max)
                    nc.vector.tensor_mul(sqy[:, 512 * cc:512 * cc + 512],
                                         yfs[:, 512 * cc:512 * cc + 512],
                                         yfs[:, 512 * cc:512 * cc + 512])
                    ynp = ynpp.tile([128, 16], FP, tag="ynp")
                    mts = [mt for mt in range(16 * cc, min(16 * cc + 16, MT))]
                    for kk, mt in enumerate(mts):
                        scg, sl = sqy_slice(mt)
                        nc.tensor.matmul(
                            ynp[:, kk:kk + 1],
                            tile_position=(32 * scg, 0),
                            lhsT=sl, rhs=nh[32 * scg:32 * scg + 32, :],
                            start=True, stop=True, skip_group_check=True)
                    nc.vector.tensor_copy(ynb[:, 16 * cc:16 * cc + len(mts)],
                                          ynp[:, 0:len(mts)])

            if use_bf:
                nc.vector.tensor_scalar(ynb2[:], ynb[:], float(EXP_S32),
                                        float(EXP_B32), op0=ALU.mult,
                                        op1=ALU.add)
            # fold e^{-|Yf|^2/2} into Z so the main-loop exp needs no bias
            ecol = big.tile([128, MT], FP)
            nc.scalar.activation(ecol[:], ynb[:], AF.Exp)
            ztf2 = big.tile([128, MT * ZP], FP)
            for mt in range(MT):
                nc.vector.tensor_scalar_mul(
                    r(ztf2[:, ZP * mt:ZP * mt + ZP]),
                    ztf[:, ZP * mt:ZP * mt + ZP],
                    ecol[:, mt:mt + 1])

            # ---------- main loop ----------
            # Both i-chunks processed per m-tile: one [128, 1024] exp per
            # tile (single bias), one weight load per mm1 pair, and two
            # long-lived accumulators.
            order = mt_order(MT)
            with (
                tc.tile_pool(name="gbuf", bufs=2, space="PSUM") as gpool,
                tc.tile_pool(name="accp", bufs=1, space="PSUM") as apool,
                tc.tile_pool(name="ebuf", bufs=3) as epool,
                tc.tile_pool(name="fin", bufs=2) as finp,
            ):
                accs = []
                for ic in range(IC):
                    acc = apool.tile([128, ICW], FP, tag=f"acc{ic}")
                    accs.append(acc)
                for k, mt in enumerate(order):
                    eng = pattern[k % len(pattern)]
                    cg, ysl = yfs_slice(mt)
                    gp = gpool.tile([128, IC * ICW], FP, tag="g")
                    for ic in range(IC):
                        nc.tensor.matmul(
                            gp[:, ICW * ic:ICW * ic + ICW],
                            tile_position=(32 * cg, 0),
                            lhsT=r(ysl),
                            rhs=r(xft[32 * cg:32 * cg + 32,
                                      ICW * ic:ICW * ic + ICW]),
                            start=True, stop=True, skip_group_check=True)
                    if eng == "A":
                        eb = epool.tile([128, IC * ICW], FP, tag="ef")
                        nc.scalar.activation(r(eb[:]), gp[:], AF.Exp)
                        lz = r(ztf2[:, ZP * mt:ZP * mt + ZP])
                        ebv = r(eb[:])
                    else:
                        # int32 Schraudolph: DVE converts (g*S+B) to int32
                        # whose bits are a valid fp32 ~= exp(g+ynb); a Pool
                        # copy re-types it as a rounded f32r producer for mm2
                        ebi = epool.tile([128, IC * ICW], I32, tag="ebi")
                        nc.vector.tensor_scalar(ebi[:], gp[:],
                                                float(EXP_S32),
                                                ynb2[:, mt:mt + 1],
                                                op0=ALU.mult, op1=ALU.add)
                        eb = epool.tile([128, IC * ICW], FP, tag="ef")
                        nc.gpsimd.tensor_copy(r(eb[:]), ebi[:].bitcast(FP))
                        lz = r(ztf[:, ZP * mt:ZP * mt + ZP])
                        ebv = r(eb[:])
                    for ic in range(IC):
                        nc.tensor.matmul(
                            accs[ic][0:ZP, :],
                            tile_position=(0, 0),
                            lhsT=lz,
                            rhs=ebv[:, ICW * ic:ICW * ic + ICW],
                            start=(k == 0), stop=(k == MT - 1),
                            skip_group_check=True)
                otm = apool.tile([128, 8 * ZP], FP, tag="ot")
                for ic in range(IC):
                    acc_s = finp.tile([ZP, ICW], FP, tag="accs")
                    nc.vector.tensor_copy(acc_s[:], accs[ic][0:ZP, :])
                    ot = otm[:, 4 * ZP * ic:4 * ZP * ic + 4 * ZP]
                    for q in range(4):
                        nc.tensor.matmul(
                            ot[:, ZP * q:ZP * q + ZP],
                            tile_position=(0, 0),
                            lhsT=acc_s[0:ZP, 128 * q:128 * q + 128],
                            rhs=ident[0:ZP, 0:ZP],
                            is_transpose=True,
                            start=(q == 0), stop=(q == 3),
                            skip_group_check=True)
                    for q in range(4):
                        rec = finp.tile([128, 1], FP, tag="rec")
                        nc.vector.reciprocal(rec[:],
                                             ot[:, ZP * q + T:ZP * q + T + 1])
                        res = finp.tile([128, T], FP, tag="res")
                        nc.vector.tensor_scalar_mul(res[:],
                                                    ot[:, ZP * q:ZP * q + T],
                                                    rec[:])
                        row = 256 * q + 128 * ic
                        (nc.sync if ic == 0 else nc.gpsimd).dma_start(
                            OUTd[row:row + 128, :], res[:])
    nc.compile()
    return nc


def make_in_maps(X, Y, Y_target, W1, b1, W2, b2, W3, b3, n_cores=N_CORES):
    import ml_dtypes

    f = lambda a: np.ascontiguousarray(np.asarray(a, dtype=np.float32))
    X, Y, Y_target = f(X), f(Y), f(Y_target)
    W1, W2, W3 = f(W1), f(W2), f(W3)
    b1, b2, b3 = f(b1), f(b2), f(b3)
    m_total = Y.shape[0]
    n_sh = X.shape[0] // n_cores
    Zf = np.zeros((m_total, ZP), np.float32)
    Zf[:, :T] = Y_target
    Zf[:, T] = 1.0
    WB = np.zeros((128, 227), np.float32)
    WB[0:64, 0:32] = W1
    WB[64:128, 0:32] = W1
    WB[:, 32:64] = np.tile(W2, (4, 1))
    WB[:, 64:96] = np.tile(W3, (4, 1))
    WB[:, 96] = np.tile(b1, 4)
    WB[:, 97] = np.tile(b2, 4)
    WB[:, 98] = np.tile(b3, 4)
    WB[:, 99:227] = np.eye(128, dtype=np.float32)
    common = dict(
        Y=Y, Zf=Zf, Zb=Zf.astype(ml_dtypes.bfloat16),
        WB=np.ascontiguousarray(WB),
    )
    return [dict(common, X=X[c * n_sh:(c + 1) * n_sh]) for c in range(n_cores)]


_NC_CACHE = {}


def _get_nc(n_sh, m_total):
    key = (n_sh, m_total)
    if key not in _NC_CACHE:
        use_f32r = os.environ.get("DKR_F32R", "1") == "1"
        _NC_CACHE[key] = build_nc(n_sh, m_total, use_f32r=use_f32r)
    return _NC_CACHE[key]


def kernel(X, Y, Y_target, W1, b1, W2, b2, W3, b3):
    from concourse.bass_utils import run_bass_kernel_spmd

    in_maps = make_in_maps(X, Y, Y_target, W1, b1, W2, b2, W3, b3)
    n_sh = in_maps[0]["X"].shape[0]
    nc = _get_nc(n_sh, np.asarray(Y).shape[0])
    res = run_bass_kernel_spmd(nc, in_maps, core_ids=list(range(N_CORES)))
    return np.concatenate([res.results[c]["out"] for c in range(N_CORES)], axis=0)
